# revision 47
# baseline (speedup 1.0000x reference)
"""Trainium2 Bass kernel for nn_AttentionBlock (column-softmax causal attention).

Reference computation (B=4, S=4096, D=128, K=64, V=128):
    Q = x @ Wq.T + bq            [B,S,64]
    Km = x @ Wk.T + bk           [B,S,64]
    Vm = x @ Wv.T + bv           [B,S,128]
    s  = Q @ Km.T / 8            [B,S,S], causal mask j>q -> -1e9
    p  = softmax(s, axis=1)      (softmax over the QUERY axis -- column softmax)
    att = p @ Vm                 [B,S,128]
    out = concat(x, att, dim=2)  [B,S,256]

Algebraic restructure (lets every matmul run bf16 with full 128-deep
contraction):
    s[q,j] = x_q M x_j^T + x_q.a + x_j.b + c   with M = Wq^T Wk / 8,
             a = Wq^T bk / 8.
    The (x_j.b + c) term is constant along the softmax (q) axis for a fixed
    column j, so it CANCELS in softmax(dim=q) and is dropped entirely.
    With G_j = M x_j^T + a (per-partition add), s^T[j,q] = sum_d G[d,j]*xT[d,q].

Flash-style column softmax: E[j,q] = exp(s^T), masked entries are 0;
l[j] = sum_q E[j,q] (ACT accumulator); att^T[v,q] = sum_j (V[j,v]/l[j])*E[j,q].
Output stays [v,q]; the HOST transposes and sums the two per-batch partials.

Sharding (8 cores): core c -> batch b = c//2, j-tile parity p = c%2.

v2 perf structure (ACT-bound design: the exp stream at (N+352)/1.2 ns +
182 ns/READ_ACC is the hard wall, ~42us/core):
  - causal mask applied ON THE PE via an identity-weights matmul that
    accumulates a -1e9 block into the scores PSUM (start=False) -- the
    vector engine is fully out of the PE->ACT critical path
  - the scalar queue carries NOTHING but the exp stream (no DMA
    descriptors, which cost ~620ns each on the issuing queue)
  - inputs spread across the sync/gpsimd/vector/tensor DMA queues in
    first-need order; outputs rotate over sync/gpsimd/vector
  - PE warmed with dummy matmuls during the input-DMA wall so the HAM
    clock gate (1.2 -> 2.4 GHz after ~3.4us of activity) flips before
    real work starts
  - final PV chunk accumulates in an open PSUM group across rows 0..15
    (no SBUF partial / merge): the post-last-row tail is 2 matmuls,
    one copy, one DMA
"""

import numpy as np

B, S, D = 4, 4096, 128
KD, VD = 64, 128
P = 128
NCORES = 8
JT = 16           # local j-tiles per core
CHUNK = 1536      # score chunk width (PSUM cols, 3 banks)

ROW_W = [S - 2 * i * P for i in range(JT)]          # E row widths
EOFF = [0] * JT
for _i in range(1, JT):
    EOFF[_i] = EOFF[_i - 1] + ROW_W[_i - 1]
ECOLS = EOFF[-1] + ROW_W[-1]                        # 34816

_CACHE = {}


def _build_program():
    from contextlib import ExitStack

    from concourse import bacc, mybir
    from concourse import tile as tile_mod

    dt = mybir.dt
    f32, bf16 = dt.float32, dt.bfloat16
    Alu = mybir.AluOpType
    ActF = mybir.ActivationFunctionType

    nc = bacc.Bacc(
        "TRN2", target_bir_lowering=False, debug=False, num_devices=NCORES
    )

    # xt: x^T with adjacent 128-col pairs SWAPPED on odd-parity cores, so
    # that core-local j-tile r always sits at columns [256r, 256r+128).
    # Scores/E/att then come out column-permuted; the host un-permutes.
    xt_d = nc.dram_tensor("xt", [P, S], bf16, kind="ExternalInput").ap()
    # s16: Mt[0:128] | WvT[128:256] | id[256:384] | mrow[384:640]
    s16_d = nc.dram_tensor("s16", [P, 640], bf16, kind="ExternalInput").ap()
    # s32: a[0] | bvb[1:129]
    s32_d = nc.dram_tensor("s32", [P, 129], f32, kind="ExternalInput").ap()
    att_d = nc.dram_tensor("att", [P, S], bf16, kind="ExternalOutput").ap()

    with tile_mod.TileContext(nc) as tc, ExitStack() as ctx:
        persist = ctx.enter_context(tc.tile_pool(name="persist", bufs=1))

        xT = persist.tile([P, S], bf16)            # [d, q'] (pair-swapped)
        GT = persist.tile([P, JT * P], bf16)       # [d, local j] = M xkv^T + a
        E_all = persist.tile([P, ECOLS], bf16)     # exp(scores^T) rows
        Vp = persist.tile([P, JT, VD], bf16)       # [j, v] scaled by 1/l
        l_all = persist.tile([P, JT], f32)
        linv = persist.tile([P, JT], f32)
        lp2 = persist.tile([P, JT], f32)           # chunk-1 l partials
        lp3 = persist.tile([P, JT], f32)           # chunk-2 l partials
        lp4 = persist.tile([P, JT], f32)           # chunk-3 l partials (row 0)
        lp5 = persist.tile([P, JT], f32)           # chunk-4 l partials (row 0)
        o7a = persist.tile([P, 512], f32)          # PV chunk-7 early partial
        V_sb = persist.tile([P, JT, VD], f32)      # V + bv, unscaled
        o5a = persist.tile([P, 512], f32)          # PV chunk-5 early partial
        o6a = persist.tile([P, 512], f32)          # PV chunk-6 early partial
        warm = persist.tile([P, 8], f32)           # exp-table warmup scratch
        scr = persist.tile([P, P], bf16)           # PE-warmup operand
        s16 = persist.tile([P, 640], bf16)
        s32 = persist.tile([P, 129], f32)
        a_sb = s32[:, 0:1]
        bvb = s32[:, 1:129]
        Mt = s16[:, 0:128]
        WvT = s16[:, 128:256]
        idm = s16[:, 256:384]
        mrow = s16[:, 384:640]

        # ---- PE/ACT warmup + input DMAs.  The scalar queue stays clean
        # (only the exp-table warmup); descriptors cost ~620ns each on the
        # issuing engine, so they're spread over sync/gpsimd/vector/tensor
        # in the order the pipeline first needs each piece.
        nc.gpsimd.memset(scr, 0.0)
        nc.gpsimd.memset(warm, 0.0)
        nc.scalar.activation(out=warm, in_=warm, func=ActF.Exp)

        nc.sync.dma_start(out=s16, in_=s16_d)
        nc.gpsimd.dma_start(out=xT[:, 2048:3072], in_=xt_d[:, 2048:3072])
        nc.scalar.dma_start(out=xT[:, 0:512], in_=xt_d[:, 0:512])
        nc.sync.dma_start(out=s32, in_=s32_d)
        nc.gpsimd.dma_start(out=xT[:, 3072:4096], in_=xt_d[:, 3072:4096])
        nc.sync.dma_start(out=xT[:, 1024:2048], in_=xt_d[:, 1024:2048])
        nc.scalar.dma_start(out=xT[:, 512:1024], in_=xt_d[:, 512:1024])

        with ExitStack() as ph:
            ps = ph.enter_context(
                tc.tile_pool(name="ps", bufs=2, space="PSUM")
            )
            aux = ph.enter_context(
                tc.tile_pool(name="aux", bufs=2, space="PSUM")
            )
            osb = ph.enter_context(tc.tile_pool(name="osb", bufs=4))

            # dummy matmuls on memset scratch: ~2us of PE activity during
            # the DMA wall flips the HAM clock gate to 2.4 GHz before the
            # real stream begins
            wps = aux.tile([P, P], f32, tag="aux", name="warm_mm")
            for _ in range(20):
                nc.tensor.matmul(
                    wps, lhsT=scr, rhs=scr, start=True, stop=True
                )

            def emit_gt(r0, r1):
                # G tiles for local j-tiles [r0, r1): tile r lives at
                # xT[:, 256r : 256r+128] under the pair-swapped layout
                pgt = aux.tile(
                    [P, (r1 - r0) * P], f32, tag="aux", name=f"gt_{r0}"
                )
                for r in range(r0, r1):
                    nc.tensor.matmul(
                        pgt[:, (r - r0) * P : (r - r0 + 1) * P],
                        lhsT=Mt,
                        rhs=xT[:, 256 * r : 256 * r + P],
                        start=True,
                        stop=True,
                    )
                nc.vector.tensor_scalar(
                    out=GT[:, r0 * P : r1 * P],
                    in0=pgt,
                    scalar1=a_sb,
                    scalar2=None,
                    op0=Alu.add,
                )

            # ---- pending PV work: thunks drained between score chunks so
            # the PV matmuls fill the PE slack while ACT owns the pace
            pending = []

            def drain(budget):
                while pending and budget > 0:
                    est, fn = pending.pop(0)
                    fn()
                    budget -= est

            def chunk_bounds(i):
                # row 0's chunks align with the input DMA pieces so each
                # activate is gated by exactly one landing transfer
                if i == 0:
                    return [0, 512, 1024, 2048, 3072, 4096]
                w = ROW_W[i]
                return list(range(0, w, CHUNK)) + [w]

            def emit_chunk(i, ci):
                bounds = chunk_bounds(i)
                q0 = 256 * i
                c0, cw = bounds[ci], bounds[ci + 1] - bounds[ci]
                sc = ps.tile([P, CHUNK], f32, tag="ps", name=f"sc_{i}_{ci}")
                for s0 in range(0, cw, 512):
                    sw = min(512, cw - s0)
                    off = q0 + c0 + s0
                    nc.tensor.matmul(
                        sc[:, s0 : s0 + sw],
                        lhsT=GT[:, i * P : (i + 1) * P],
                        rhs=xT[:, off : off + sw],
                        start=True,
                        stop=True,
                    )
                    if ci == 0 and s0 == 0:
                        # accumulate the -1e9 diagonal mask block on
                        # the PE itself (identity weights, mask rhs)
                        nc.tensor.matmul(
                            sc[:, : 2 * P],
                            lhsT=idm,
                            rhs=mrow,
                            start=False,
                            stop=True,
                            skip_group_check=True,
                        )
                ecol = EOFF[i] + c0
                nc.scalar.activation(
                    out=E_all[:, ecol : ecol + cw],
                    in_=sc[:, :cw],
                    func=ActF.Exp,
                    accum_out=[l_all, lp2, lp3, lp4, lp5][ci][:, i : i + 1],
                )
                b = int(0.55 * cw) + 280
                if i in (3, 5, 7) and ci == 0:
                    b -= 700
                drain(b)

            def finish_row(i):
                nch = len(chunk_bounds(i)) - 1
                for pp in ([lp2, lp3, lp4, lp5][: nch - 1]):
                    nc.vector.tensor_tensor(
                        out=l_all[:, i : i + 1],
                        in0=l_all[:, i : i + 1],
                        in1=pp[:, i : i + 1],
                        op=Alu.add,
                    )
                nc.vector.reciprocal(linv[:, i : i + 1], l_all[:, i : i + 1])
                nc.vector.tensor_scalar(
                    out=Vp[:, i, :],
                    in0=V_sb[:, i, :],
                    scalar1=linv[:, i : i + 1],
                    scalar2=None,
                    op0=Alu.mult,
                )

            def emit_qk_row(i):
                for ci in range(len(chunk_bounds(i)) - 1):
                    emit_chunk(i, ci)
                finish_row(i)

            def emit_v(i):
                # V projection for tile i (runs at startup; needs no l)
                pv = aux.tile([P, VD], f32, tag="aux", name=f"v_{i}")
                nc.tensor.matmul(
                    pv,
                    lhsT=xT[:, 256 * i : 256 * i + P],
                    rhs=WvT,
                    start=True,
                    stop=True,
                )
                nc.vector.tensor_tensor(
                    out=V_sb[:, i, :], in0=pv, in1=bvb, op=Alu.add
                )

            OUTQ = [nc.sync, nc.gpsimd]

            def emit_out(c, ap, merge=None):
                ob = osb.tile([P, 512], bf16, tag="osb", name=f"osb_{c}")
                if merge is None:
                    nc.vector.tensor_copy(ob, ap)
                else:
                    nc.vector.tensor_tensor(
                        out=ob, in0=ap, in1=merge, op=Alu.add
                    )
                OUTQ[c % 2].dma_start(
                    out=att_d[:, c * 512 : (c + 1) * 512], in_=ob
                )

            class PvGroup:
                """PSUM accumulation over rows for output cols
                [g0, g0+gw); MMs are enqueued as thunks and drained
                between score chunks."""

                def __init__(self, name, g0, gw, shared=None, half=None):
                    self.name, self.g0, self.gw = name, g0, gw
                    self.shared, self.half = shared, half
                    self.ap = None
                    self.started = False

                def _ap(self):
                    if self.shared is not None:
                        if self.shared.get("ap") is None:
                            self.shared["ap"] = aux.tile(
                                [P, 512], f32, tag="aux", name=self.name
                            )
                        full = self.shared["ap"]
                        h = self.half
                        return full[:, h * 256 : h * 256 + 256]
                    if self.ap is None:
                        self.ap = aux.tile(
                            [P, self.gw], f32, tag="aux", name=self.name
                        )
                    return self.ap

                def enq(self, rows, last=False):
                    rows = list(rows)
                    for k, ii in enumerate(rows):
                        pending.append(
                            self._mk(ii, last and k == len(rows) - 1)
                        )

                def _mk(self, ii, is_last):
                    g0, gw = self.g0, self.gw
                    lo_q = max(g0, 256 * ii)
                    n = g0 + gw - lo_q
                    ecol = EOFF[ii] + lo_q - 256 * ii
                    p0 = lo_q - g0

                    def fn():
                        ap = self._ap()
                        st = not self.started
                        self.started = True
                        nc.tensor.matmul(
                            ap[:, p0 : p0 + n],
                            lhsT=Vp[:, ii, :],
                            rhs=E_all[:, ecol : ecol + n],
                            start=st,
                            stop=is_last,
                            skip_group_check=True,
                        )

                    return (int(n * 0.42) + 25, fn)

                def fin(self, mode, other=None, lo=0, hi=None):
                    # mode: "out" -> copy+DMA, "save" -> copy to SBUF
                    # partial, "merge" -> add SBUF partial, then DMA.
                    # lo/hi select a column sub-range of the group.
                    def fn():
                        h = self.gw if hi is None else hi
                        ap = self._ap()[:, lo:h]
                        g0, gw = self.g0 + lo, h - lo
                        if mode == "save":
                            nc.vector.tensor_copy(other, ap)
                            return
                        ob = osb.tile(
                            [P, gw], bf16, tag="osb", name=f"ob_{self.name}_{lo}"
                        )
                        if mode == "merge":
                            nc.vector.tensor_tensor(
                                out=ob, in0=ap, in1=other, op=Alu.add
                            )
                        else:
                            nc.vector.tensor_copy(ob, ap)
                        OUTQ[(g0 // 512 + (g0 % 512) // 256) % 2].dma_start(
                            out=att_d[:, g0 : g0 + gw], in_=ob
                        )

                    pending.append((80, fn))

            g7t = PvGroup("pv7t", 3584, 512)
            g5p = PvGroup("pv5p", 2560, 512)
            g6p = PvGroup("pv6p", 3072, 512)
            g7p = PvGroup("pv7p", 3584, 512)
            gful = {}

            # ---- early phase: row 0's chunks emitted in the order their
            # xT pieces land (piece 3 arrives on the lightly-loaded gpsimd
            # ring before piece 2), so the exp stream never starves while
            # the 1MB of x^T is still in flight
            emit_gt(0, 1)
            emit_v(0)
            emit_chunk(0, 0)
            emit_gt(1, 3)
            emit_v(1)
            emit_v(2)
            for ci in (1, 3, 2, 4):
                emit_chunk(0, ci)
            finish_row(0)

            for i in range(1, JT):
                if i == 1:
                    emit_gt(3, 7)
                    emit_v(3)
                if i == 3:
                    emit_gt(7, 11)
                    for t in range(4, 8):
                        emit_v(t)
                if i == 5:
                    emit_gt(11, 14)
                    for t in range(8, 12):
                        emit_v(t)
                if i == 7:
                    emit_gt(14, 16)
                    for t in range(12, 16):
                        emit_v(t)
                emit_qk_row(i)
                # PV schedule: every chunk opens as soon as most of its
                # rows exist and closes two rows later (open PSUM groups,
                # FIFO drain order keeps at most 2 groups live); SBUF
                # partials front-load chunks 5-7; chunk 7's last rows
                # accumulate in an open group so only row 15's MM + one
                # merge trail the exp stream
                if i == 1:
                    gful[0] = PvGroup("pv0", 0, 512)
                    gful[0].enq(range(0, 2), last=True)
                    gful[0].fin("out")
                    gful[1] = PvGroup("pv1", 512, 512)
                    gful[1].enq(range(0, 2))
                if i == 3:
                    gful[1].enq(range(2, 4), last=True)
                    gful[1].fin("out")
                    gful[2] = PvGroup("pv2", 1024, 512)
                    gful[2].enq(range(0, 4))
                if i == 5:
                    gful[2].enq(range(4, 6), last=True)
                    gful[2].fin("out")
                    gful[3] = PvGroup("pv3", 1536, 512)
                    gful[3].enq(range(0, 6))
                    g5p.enq(range(0, 6))
                if i == 7:
                    gful[3].enq(range(6, 8), last=True)
                    gful[3].fin("out")
                    g5p.enq(range(6, 8), last=True)
                    g5p.fin("save", o5a)
                    gful[4] = PvGroup("pv4", 2048, 512)
                    gful[4].enq(range(0, 8))
                    g6p.enq(range(0, 8))
                if i == 9:
                    gful[4].enq(range(8, 10), last=True)
                    gful[4].fin("out")
                    g6p.enq(range(8, 10), last=True)
                    g6p.fin("save", o6a)
                    g7p.enq(range(0, 10))
                if i == 11:
                    g7p.enq(range(10, 12), last=True)
                    g7p.fin("save", o7a)
                    g5t = PvGroup("pv5t", 2560, 512)
                    g5t.enq(range(8, 12), last=True)
                    g5t.fin("merge", o5a)
                    g6t = PvGroup("pv6t", 3072, 512)
                    g6t.enq(range(10, 12))
                if i == 13:
                    g6t.enq(range(12, 14), last=True)
                    g6t.fin("merge", o6a)
                    g7t.enq(range(12, 14))
                if i == 14:
                    # after row 14, output cols [3584:3840] are final
                    g7t.enq([14])
                    g7t.fin("merge", o7a[:, 0:256], lo=0, hi=256)
                if i == 15:
                    g7t.enq([15], last=True)
                    g7t.fin("merge", o7a[:, 256:512], lo=256, hi=512)
            drain(10**9)

    nc.compile()
    return nc


def _host_inputs(x, Wq, bq, Wk, bk, Wv, bv):
    """Per-core input maps (host does layout prep + tiny precomputes)."""
    import ml_dtypes

    hf = ml_dtypes.bfloat16
    x_full = np.ascontiguousarray(x, dtype=np.float32)
    Wq = np.asarray(Wq, np.float32)
    Wk = np.asarray(Wk, np.float32)
    bk = np.asarray(bk, np.float32)
    Wv = np.asarray(Wv, np.float32)
    bv = np.asarray(bv, np.float32)

    M = (Wq.T @ Wk) / 8.0                      # [D, D]
    Mt = np.ascontiguousarray(M.T).astype(hf)
    a = ((Wq.T @ bk) / 8.0).reshape(D, 1)      # [D, 1]
    WvT = np.ascontiguousarray(Wv.T).astype(hf)
    bvb = np.tile(bv.reshape(1, VD), (P, 1))   # [P, V]
    idm = np.eye(P, dtype=np.float32)

    # mask row: diagonal tile is ALWAYS the first 128 cols of a row under
    # the pair-swapped layout; for p=1 the second 128 cols are the
    # lower-numbered global tile -> fully masked
    tri = np.where(
        np.arange(P)[None, :] >= np.arange(P)[:, None], 0.0, -1e9
    ).astype(np.float32)
    mrows = []
    for p in (0, 1):
        m = np.zeros((P, 2 * P), np.float32)
        m[:, :P] = tri
        if p == 1:
            m[:, P:] = -1e9
        mrows.append(m)

    s16s = [
        np.ascontiguousarray(
            np.concatenate([Mt, WvT, idm.astype(hf), mrows[p].astype(hf)],
                           axis=1)
        )
        for p in (0, 1)
    ]
    s32 = np.ascontiguousarray(
        np.concatenate([a, bvb], axis=1).astype(np.float32)
    )
    # per-parity xT: odd cores get adjacent 128-col pairs swapped so local
    # j-tile r sits at columns [256r, 256r+128) on every core
    sw = np.arange(S // P).reshape(-1, 2)[:, ::-1].reshape(-1)
    xts = []
    for b in range(B):
        xt = np.ascontiguousarray(x_full[b].T.astype(hf))
        xts.append(
            (
                xt,
                np.ascontiguousarray(
                    xt.reshape(P, S // P, P)[:, sw, :].reshape(P, S)
                ),
            )
        )
    in_maps = []
    for c in range(NCORES):
        b, p = c // 2, c % 2
        in_maps.append(
            {
                "xt": xts[b][p],
                "s16": s16s[p],
                "s32": s32,
            }
        )
    return in_maps


def _get_program():
    if "nc" not in _CACHE:
        _CACHE["nc"] = _build_program()
    return _CACHE["nc"]


def run_on_device(in_maps, trace=False, trace_kwargs=None):
    from concourse import bass_utils

    nc = _get_program()
    return bass_utils.run_bass_kernel_spmd(
        nc,
        in_maps,
        core_ids=list(range(NCORES)),
        trace=trace,
        trace_kwargs=trace_kwargs or {},
    )


def kernel(x, Wq, bq, Wk, bk, Wv, bv):
    x = np.asarray(x, np.float32)
    in_maps = _host_inputs(x, Wq, bq, Wk, bk, Wv, bv)
    res = run_on_device(in_maps)
    sw = np.arange(S // P).reshape(-1, 2)[:, ::-1].reshape(-1)
    out = np.empty((B, S, D + VD), np.float32)
    for b in range(B):
        a1 = res.results[2 * b + 1]["att"].astype(np.float32)
        # odd core's att columns are pair-swapped; undo before the add
        a1 = a1.reshape(P, S // P, P)[:, sw, :].reshape(P, S)
        attT = res.results[2 * b]["att"].astype(np.float32) + a1
        out[b, :, :D] = x[b]
        out[b, :, D:] = attT.T
    return out


# revision 48
# speedup vs baseline: 1.0149x; 1.0149x over previous
"""Trainium2 Bass kernel for nn_AttentionBlock (column-softmax causal attention).

Reference computation (B=4, S=4096, D=128, K=64, V=128):
    Q = x @ Wq.T + bq            [B,S,64]
    Km = x @ Wk.T + bk           [B,S,64]
    Vm = x @ Wv.T + bv           [B,S,128]
    s  = Q @ Km.T / 8            [B,S,S], causal mask j>q -> -1e9
    p  = softmax(s, axis=1)      (softmax over the QUERY axis -- column softmax)
    att = p @ Vm                 [B,S,128]
    out = concat(x, att, dim=2)  [B,S,256]

Algebraic restructure (lets every matmul run bf16 with full 128-deep
contraction):
    s[q,j] = x_q M x_j^T + x_q.a + x_j.b + c   with M = Wq^T Wk / 8,
             a = Wq^T bk / 8.
    The (x_j.b + c) term is constant along the softmax (q) axis for a fixed
    column j, so it CANCELS in softmax(dim=q) and is dropped entirely.
    With G_j = M x_j^T + a (per-partition add), s^T[j,q] = sum_d G[d,j]*xT[d,q].

Flash-style column softmax: E[j,q] = exp(s^T), masked entries are 0;
l[j] = sum_q E[j,q] (ACT accumulator); att^T[v,q] = sum_j (V[j,v]/l[j])*E[j,q].
Output stays [v,q]; the HOST transposes and sums the two per-batch partials.

Sharding (8 cores): core c -> batch b = c//2, j-tile parity p = c%2.

Perf structure (ACT-bound: the exp stream at (N+352)/1.2 ns + 182
ns/READ_ACC per chunk is the hard wall, ~42us/core; PE warm ~38us):
  - NO xkvt input: the host pair-swaps adjacent 128-col tiles of x^T on
    odd cores so local j-tile r sits at xT[:, 256r:256r+128] on every
    core (SPMD-uniform); saves 512KB of the input-DMA wall.  The odd
    cores' att output comes back column-permuted; the host un-permutes.
  - causal mask applied ON THE PE via an identity-weights matmul that
    accumulates a -1e9 block into the scores PSUM (start=False) -- the
    vector engine is fully out of the PE->ACT critical path
  - the scalar queue carries only the exp stream + the two earliest xT
    DMA descriptors (issued before the stream starts)
  - row 0 is split into 5 chunks emitted in input-piece-landing order,
    so the exp stream starts as soon as the first 128KB piece lands
  - PE warmed with dummy matmuls during the input-DMA wall so the HAM
    clock gate (1.2 -> 2.4 GHz after ~3.4us of activity) flips before
    real work starts
  - PV matmuls run as a pending-thunk queue drained with a time budget
    after every activate, so they fill PE slack under the ACT stream;
    every output chunk opens as an early PSUM group (or SBUF partial
    for chunks 5-7) and closes 2 rows later; chunk 7's high half closes
    after row 15's single 256-col matmul + one merge
"""

import numpy as np

B, S, D = 4, 4096, 128
KD, VD = 64, 128
P = 128
NCORES = 8
JT = 16           # local j-tiles per core
CHUNK = 1536      # score chunk width (PSUM cols, 3 banks)

ROW_W = [S - 2 * i * P for i in range(JT)]          # E row widths
EOFF = [0] * JT
for _i in range(1, JT):
    EOFF[_i] = EOFF[_i - 1] + ROW_W[_i - 1]
ECOLS = EOFF[-1] + ROW_W[-1]                        # 34816

_CACHE = {}


def _build_program():
    from contextlib import ExitStack

    from concourse import bacc, mybir
    from concourse import tile as tile_mod

    dt = mybir.dt
    f32, bf16 = dt.float32, dt.bfloat16
    Alu = mybir.AluOpType
    ActF = mybir.ActivationFunctionType

    nc = bacc.Bacc(
        "TRN2", target_bir_lowering=False, debug=False, num_devices=NCORES
    )

    # xt: x^T with adjacent 128-col pairs SWAPPED on odd-parity cores, so
    # that core-local j-tile r always sits at columns [256r, 256r+128).
    # Scores/E/att then come out column-permuted; the host un-permutes.
    xt_d = nc.dram_tensor("xt", [P, S], bf16, kind="ExternalInput").ap()
    # s16: Mt[0:128] | WvT[128:256] | id[256:384] | mrow[384:640]
    s16_d = nc.dram_tensor("s16", [P, 640], bf16, kind="ExternalInput").ap()
    # s32: a[0] | bvb[1:129]
    s32_d = nc.dram_tensor("s32", [P, 129], f32, kind="ExternalInput").ap()
    att_d = nc.dram_tensor("att", [P, S], bf16, kind="ExternalOutput").ap()

    with tile_mod.TileContext(nc) as tc, ExitStack() as ctx:
        persist = ctx.enter_context(tc.tile_pool(name="persist", bufs=1))

        xT = persist.tile([P, S], bf16)            # [d, q'] (pair-swapped)
        GT = persist.tile([P, JT * P], bf16)       # [d, local j] = M xkv^T + a
        E_all = persist.tile([P, ECOLS], bf16)     # exp(scores^T) rows
        Vp = persist.tile([P, JT, VD], bf16)       # [j, v] scaled by 1/l
        l_all = persist.tile([P, JT], f32)
        linv = persist.tile([P, JT], f32)
        lp2 = persist.tile([P, JT], f32)           # chunk-1 l partials
        lp3 = persist.tile([P, JT], f32)           # chunk-2 l partials
        lp4 = persist.tile([P, JT], f32)           # chunk-3 l partials (row 0)
        lp5 = persist.tile([P, JT], f32)           # chunk-4 l partials (row 0)
        o7a = persist.tile([P, 512], f32)          # PV chunk-7 early partial
        V_sb = persist.tile([P, JT, VD], f32)      # V + bv, unscaled
        o5a = persist.tile([P, 512], f32)          # PV chunk-5 early partial
        o6a = persist.tile([P, 512], f32)          # PV chunk-6 early partial
        warm = persist.tile([P, 8], f32)           # exp-table warmup scratch
        scr = persist.tile([P, P], bf16)           # PE-warmup operand
        s16 = persist.tile([P, 640], bf16)
        s32 = persist.tile([P, 129], f32)
        a_sb = s32[:, 0:1]
        bvb = s32[:, 1:129]
        Mt = s16[:, 0:128]
        WvT = s16[:, 128:256]
        idm = s16[:, 256:384]
        mrow = s16[:, 384:640]

        # ---- PE/ACT warmup + input DMAs.  The scalar queue stays clean
        # (only the exp-table warmup); descriptors cost ~620ns each on the
        # issuing engine, so they're spread over sync/gpsimd/vector/tensor
        # in the order the pipeline first needs each piece.
        nc.gpsimd.memset(scr, 0.0)
        nc.gpsimd.memset(warm, 0.0)
        nc.scalar.activation(out=warm, in_=warm, func=ActF.Exp)

        nc.sync.dma_start(out=s16, in_=s16_d)
        nc.gpsimd.dma_start(out=xT[:, 2048:3072], in_=xt_d[:, 2048:3072])
        nc.scalar.dma_start(out=xT[:, 0:512], in_=xt_d[:, 0:512])
        nc.sync.dma_start(out=s32, in_=s32_d)
        nc.gpsimd.dma_start(out=xT[:, 3072:4096], in_=xt_d[:, 3072:4096])
        nc.sync.dma_start(out=xT[:, 1024:2048], in_=xt_d[:, 1024:2048])
        nc.scalar.dma_start(out=xT[:, 512:1024], in_=xt_d[:, 512:1024])

        with ExitStack() as ph:
            ps = ph.enter_context(
                tc.tile_pool(name="ps", bufs=2, space="PSUM")
            )
            aux = ph.enter_context(
                tc.tile_pool(name="aux", bufs=2, space="PSUM")
            )
            osb = ph.enter_context(tc.tile_pool(name="osb", bufs=4))

            # dummy matmuls on memset scratch: ~2us of PE activity during
            # the DMA wall flips the HAM clock gate to 2.4 GHz before the
            # real stream begins
            wps = aux.tile([P, P], f32, tag="aux", name="warm_mm")
            for _ in range(20):
                nc.tensor.matmul(
                    wps, lhsT=scr, rhs=scr, start=True, stop=True
                )

            def emit_gt(r0, r1):
                # G tiles for local j-tiles [r0, r1): tile r lives at
                # xT[:, 256r : 256r+128] under the pair-swapped layout
                pgt = aux.tile(
                    [P, (r1 - r0) * P], f32, tag="aux", name=f"gt_{r0}"
                )
                for r in range(r0, r1):
                    nc.tensor.matmul(
                        pgt[:, (r - r0) * P : (r - r0 + 1) * P],
                        lhsT=Mt,
                        rhs=xT[:, 256 * r : 256 * r + P],
                        start=True,
                        stop=True,
                    )
                nc.vector.tensor_scalar(
                    out=GT[:, r0 * P : r1 * P],
                    in0=pgt,
                    scalar1=a_sb,
                    scalar2=None,
                    op0=Alu.add,
                )

            # ---- pending PV work: thunks drained between score chunks so
            # the PV matmuls fill the PE slack while ACT owns the pace
            pending = []

            def drain(budget):
                while pending and budget > 0:
                    est, fn = pending.pop(0)
                    fn()
                    budget -= est

            def chunk_bounds(i):
                # row 0's chunks align with the input DMA pieces so each
                # activate is gated by exactly one landing transfer
                if i == 0:
                    return [0, 512, 1024, 2048, 3072, 4096]
                w = ROW_W[i]
                return list(range(0, w, CHUNK)) + [w]

            def emit_chunk(i, ci):
                bounds = chunk_bounds(i)
                q0 = 256 * i
                c0, cw = bounds[ci], bounds[ci + 1] - bounds[ci]
                sc = ps.tile([P, CHUNK], f32, tag="ps", name=f"sc_{i}_{ci}")
                for s0 in range(0, cw, 512):
                    sw = min(512, cw - s0)
                    off = q0 + c0 + s0
                    nc.tensor.matmul(
                        sc[:, s0 : s0 + sw],
                        lhsT=GT[:, i * P : (i + 1) * P],
                        rhs=xT[:, off : off + sw],
                        start=True,
                        stop=True,
                    )
                    if ci == 0 and s0 == 0:
                        # accumulate the -1e9 diagonal mask block on
                        # the PE itself (identity weights, mask rhs)
                        nc.tensor.matmul(
                            sc[:, : 2 * P],
                            lhsT=idm,
                            rhs=mrow,
                            start=False,
                            stop=True,
                            skip_group_check=True,
                        )
                ecol = EOFF[i] + c0
                nc.scalar.activation(
                    out=E_all[:, ecol : ecol + cw],
                    in_=sc[:, :cw],
                    func=ActF.Exp,
                    accum_out=[l_all, lp2, lp3, lp4, lp5][ci][:, i : i + 1],
                )
                b = int(0.55 * cw) + 280
                if i in (3, 5, 7) and ci == 0:
                    b -= 700
                drain(b)

            def finish_row(i):
                nch = len(chunk_bounds(i)) - 1
                for pp in ([lp2, lp3, lp4, lp5][: nch - 1]):
                    nc.vector.tensor_tensor(
                        out=l_all[:, i : i + 1],
                        in0=l_all[:, i : i + 1],
                        in1=pp[:, i : i + 1],
                        op=Alu.add,
                    )
                nc.vector.reciprocal(linv[:, i : i + 1], l_all[:, i : i + 1])
                nc.vector.tensor_scalar(
                    out=Vp[:, i, :],
                    in0=V_sb[:, i, :],
                    scalar1=linv[:, i : i + 1],
                    scalar2=None,
                    op0=Alu.mult,
                )

            def emit_qk_row(i):
                for ci in range(len(chunk_bounds(i)) - 1):
                    emit_chunk(i, ci)
                finish_row(i)

            def emit_v(i):
                # V projection for tile i (runs at startup; needs no l)
                pv = aux.tile([P, VD], f32, tag="aux", name=f"v_{i}")
                nc.tensor.matmul(
                    pv,
                    lhsT=xT[:, 256 * i : 256 * i + P],
                    rhs=WvT,
                    start=True,
                    stop=True,
                )
                nc.vector.tensor_tensor(
                    out=V_sb[:, i, :], in0=pv, in1=bvb, op=Alu.add
                )

            OUTQ = [nc.sync, nc.gpsimd]

            def emit_out(c, ap, merge=None):
                ob = osb.tile([P, 512], bf16, tag="osb", name=f"osb_{c}")
                if merge is None:
                    nc.vector.tensor_copy(ob, ap)
                else:
                    nc.vector.tensor_tensor(
                        out=ob, in0=ap, in1=merge, op=Alu.add
                    )
                OUTQ[c % 2].dma_start(
                    out=att_d[:, c * 512 : (c + 1) * 512], in_=ob
                )

            class PvGroup:
                """PSUM accumulation over rows for output cols
                [g0, g0+gw); MMs are enqueued as thunks and drained
                between score chunks."""

                def __init__(self, name, g0, gw, shared=None, half=None):
                    self.name, self.g0, self.gw = name, g0, gw
                    self.shared, self.half = shared, half
                    self.ap = None
                    self.started = False

                def _ap(self):
                    if self.shared is not None:
                        if self.shared.get("ap") is None:
                            self.shared["ap"] = aux.tile(
                                [P, 512], f32, tag="aux", name=self.name
                            )
                        full = self.shared["ap"]
                        h = self.half
                        return full[:, h * 256 : h * 256 + 256]
                    if self.ap is None:
                        self.ap = aux.tile(
                            [P, self.gw], f32, tag="aux", name=self.name
                        )
                    return self.ap

                def enq(self, rows, last=False):
                    rows = list(rows)
                    for k, ii in enumerate(rows):
                        pending.append(
                            self._mk(ii, last and k == len(rows) - 1)
                        )

                def _mk(self, ii, is_last):
                    g0, gw = self.g0, self.gw
                    lo_q = max(g0, 256 * ii)
                    n = g0 + gw - lo_q
                    ecol = EOFF[ii] + lo_q - 256 * ii
                    p0 = lo_q - g0

                    def fn():
                        ap = self._ap()
                        st = not self.started
                        self.started = True
                        nc.tensor.matmul(
                            ap[:, p0 : p0 + n],
                            lhsT=Vp[:, ii, :],
                            rhs=E_all[:, ecol : ecol + n],
                            start=st,
                            stop=is_last,
                            skip_group_check=True,
                        )

                    return (int(n * 0.42) + 25, fn)

                def fin(self, mode, other=None, lo=0, hi=None):
                    # mode: "out" -> copy+DMA, "save" -> copy to SBUF
                    # partial, "merge" -> add SBUF partial, then DMA.
                    # lo/hi select a column sub-range of the group.
                    def fn():
                        h = self.gw if hi is None else hi
                        ap = self._ap()[:, lo:h]
                        g0, gw = self.g0 + lo, h - lo
                        if mode == "save":
                            nc.vector.tensor_copy(other, ap)
                            return
                        ob = osb.tile(
                            [P, gw], bf16, tag="osb", name=f"ob_{self.name}_{lo}"
                        )
                        if mode == "merge":
                            nc.vector.tensor_tensor(
                                out=ob, in0=ap, in1=other, op=Alu.add
                            )
                        else:
                            nc.vector.tensor_copy(ob, ap)
                        OUTQ[(g0 // 512 + (g0 % 512) // 256) % 2].dma_start(
                            out=att_d[:, g0 : g0 + gw], in_=ob
                        )

                    pending.append((80, fn))

            g7t = PvGroup("pv7t", 3584, 512)
            g5p = PvGroup("pv5p", 2560, 512)
            g6p = PvGroup("pv6p", 3072, 512)
            g7p = PvGroup("pv7p", 3584, 512)
            gful = {}

            # ---- early phase: row 0's chunks emitted in the order their
            # xT pieces land (piece 3 arrives on the lightly-loaded gpsimd
            # ring before piece 2), so the exp stream never starves while
            # the 1MB of x^T is still in flight
            emit_gt(0, 1)
            emit_v(0)
            emit_chunk(0, 0)
            emit_gt(1, 3)
            emit_v(1)
            emit_v(2)
            for ci in (1, 3, 2, 4):
                emit_chunk(0, ci)
            finish_row(0)

            for i in range(1, JT):
                if i == 1:
                    emit_gt(3, 7)
                    emit_v(3)
                if i == 3:
                    emit_gt(7, 11)
                    for t in range(4, 8):
                        emit_v(t)
                if i == 5:
                    emit_gt(11, 14)
                    for t in range(8, 12):
                        emit_v(t)
                if i == 7:
                    emit_gt(14, 16)
                    for t in range(12, 16):
                        emit_v(t)
                emit_qk_row(i)
                # PV schedule: every chunk opens as soon as most of its
                # rows exist and closes two rows later (open PSUM groups,
                # FIFO drain order keeps at most 2 groups live); SBUF
                # partials front-load chunks 5-7; chunk 7's last rows
                # accumulate in an open group so only row 15's MM + one
                # merge trail the exp stream
                if i == 1:
                    gful[0] = PvGroup("pv0", 0, 512)
                    gful[0].enq(range(0, 2), last=True)
                    gful[0].fin("out")
                    gful[1] = PvGroup("pv1", 512, 512)
                    gful[1].enq(range(0, 2))
                if i == 3:
                    gful[1].enq(range(2, 4), last=True)
                    gful[1].fin("out")
                    gful[2] = PvGroup("pv2", 1024, 512)
                    gful[2].enq(range(0, 4))
                if i == 5:
                    gful[2].enq(range(4, 6), last=True)
                    gful[2].fin("out")
                    gful[3] = PvGroup("pv3", 1536, 512)
                    gful[3].enq(range(0, 6))
                    g5p.enq(range(0, 6))
                if i == 7:
                    gful[3].enq(range(6, 8), last=True)
                    gful[3].fin("out")
                    g5p.enq(range(6, 8), last=True)
                    g5p.fin("save", o5a)
                    gful[4] = PvGroup("pv4", 2048, 512)
                    gful[4].enq(range(0, 8))
                    g6p.enq(range(0, 8))
                if i == 9:
                    gful[4].enq(range(8, 10), last=True)
                    gful[4].fin("out")
                    g6p.enq(range(8, 10), last=True)
                    g6p.fin("save", o6a)
                    g7p.enq(range(0, 10))
                if i == 11:
                    g7p.enq(range(10, 12), last=True)
                    g7p.fin("save", o7a)
                    g5t = PvGroup("pv5t", 2560, 512)
                    g5t.enq(range(8, 12), last=True)
                    g5t.fin("merge", o5a)
                    g6t = PvGroup("pv6t", 3072, 512)
                    g6t.enq(range(10, 12))
                if i == 13:
                    g6t.enq(range(12, 14), last=True)
                    g6t.fin("merge", o6a)
                    g7t.enq(range(12, 14))
                if i == 14:
                    # after row 14, output cols [3584:3840] are final
                    g7t.enq([14])
                    g7t.fin("merge", o7a[:, 0:256], lo=0, hi=256)
                if i == 15:
                    g7t.enq([15], last=True)
                    g7t.fin("merge", o7a[:, 256:512], lo=256, hi=512)
            drain(10**9)

    nc.compile()
    return nc


def _host_inputs(x, Wq, bq, Wk, bk, Wv, bv):
    """Per-core input maps (host does layout prep + tiny precomputes)."""
    import ml_dtypes

    hf = ml_dtypes.bfloat16
    x_full = np.ascontiguousarray(x, dtype=np.float32)
    Wq = np.asarray(Wq, np.float32)
    Wk = np.asarray(Wk, np.float32)
    bk = np.asarray(bk, np.float32)
    Wv = np.asarray(Wv, np.float32)
    bv = np.asarray(bv, np.float32)

    M = (Wq.T @ Wk) / 8.0                      # [D, D]
    Mt = np.ascontiguousarray(M.T).astype(hf)
    a = ((Wq.T @ bk) / 8.0).reshape(D, 1)      # [D, 1]
    WvT = np.ascontiguousarray(Wv.T).astype(hf)
    bvb = np.tile(bv.reshape(1, VD), (P, 1))   # [P, V]
    idm = np.eye(P, dtype=np.float32)

    # mask row: diagonal tile is ALWAYS the first 128 cols of a row under
    # the pair-swapped layout; for p=1 the second 128 cols are the
    # lower-numbered global tile -> fully masked
    tri = np.where(
        np.arange(P)[None, :] >= np.arange(P)[:, None], 0.0, -1e9
    ).astype(np.float32)
    mrows = []
    for p in (0, 1):
        m = np.zeros((P, 2 * P), np.float32)
        m[:, :P] = tri
        if p == 1:
            m[:, P:] = -1e9
        mrows.append(m)

    s16s = [
        np.ascontiguousarray(
            np.concatenate([Mt, WvT, idm.astype(hf), mrows[p].astype(hf)],
                           axis=1)
        )
        for p in (0, 1)
    ]
    s32 = np.ascontiguousarray(
        np.concatenate([a, bvb], axis=1).astype(np.float32)
    )
    # per-parity xT: odd cores get adjacent 128-col pairs swapped so local
    # j-tile r sits at columns [256r, 256r+128) on every core
    sw = np.arange(S // P).reshape(-1, 2)[:, ::-1].reshape(-1)
    xts = []
    for b in range(B):
        xt = np.ascontiguousarray(x_full[b].T.astype(hf))
        xts.append(
            (
                xt,
                np.ascontiguousarray(
                    xt.reshape(P, S // P, P)[:, sw, :].reshape(P, S)
                ),
            )
        )
    in_maps = []
    for c in range(NCORES):
        b, p = c // 2, c % 2
        in_maps.append(
            {
                "xt": xts[b][p],
                "s16": s16s[p],
                "s32": s32,
            }
        )
    return in_maps


def _get_program():
    if "nc" not in _CACHE:
        _CACHE["nc"] = _build_program()
    return _CACHE["nc"]


def run_on_device(in_maps, trace=False, trace_kwargs=None):
    from concourse import bass_utils

    nc = _get_program()
    return bass_utils.run_bass_kernel_spmd(
        nc,
        in_maps,
        core_ids=list(range(NCORES)),
        trace=trace,
        trace_kwargs=trace_kwargs or {},
    )


def kernel(x, Wq, bq, Wk, bk, Wv, bv):
    x = np.asarray(x, np.float32)
    in_maps = _host_inputs(x, Wq, bq, Wk, bk, Wv, bv)
    res = run_on_device(in_maps)
    sw = np.arange(S // P).reshape(-1, 2)[:, ::-1].reshape(-1)
    out = np.empty((B, S, D + VD), np.float32)
    for b in range(B):
        a1 = res.results[2 * b + 1]["att"].astype(np.float32)
        # odd core's att columns are pair-swapped; undo before the add
        a1 = a1.reshape(P, S // P, P)[:, sw, :].reshape(P, S)
        attT = res.results[2 * b]["att"].astype(np.float32) + a1
        out[b, :, :D] = x[b]
        out[b, :, D:] = attT.T
    return out


# revision 49
# speedup vs baseline: 1.0163x; 1.0014x over previous
"""Trainium2 Bass kernel for nn_AttentionBlock (column-softmax causal attention).

Reference computation (B=4, S=4096, D=128, K=64, V=128):
    Q = x @ Wq.T + bq            [B,S,64]
    Km = x @ Wk.T + bk           [B,S,64]
    Vm = x @ Wv.T + bv           [B,S,128]
    s  = Q @ Km.T / 8            [B,S,S], causal mask j>q -> -1e9
    p  = softmax(s, axis=1)      (softmax over the QUERY axis -- column softmax)
    att = p @ Vm                 [B,S,128]
    out = concat(x, att, dim=2)  [B,S,256]

Algebraic restructure (lets every matmul run bf16 with full 128-deep
contraction):
    s[q,j] = x_q M x_j^T + x_q.a + x_j.b + c   with M = Wq^T Wk / 8,
             a = Wq^T bk / 8.
    The (x_j.b + c) term is constant along the softmax (q) axis for a fixed
    column j, so it CANCELS in softmax(dim=q) and is dropped entirely.
    With G_j = M x_j^T + a (per-partition add), s^T[j,q] = sum_d G[d,j]*xT[d,q].

Flash-style column softmax: E[j,q] = exp(s^T), masked entries are 0;
l[j] = sum_q E[j,q] (ACT accumulator); att^T[v,q] = sum_j (V[j,v]/l[j])*E[j,q].
Output stays [v,q]; the HOST transposes and sums the two per-batch partials.

Sharding (8 cores): core c -> batch b = c//2, j-tile parity p = c%2.

Perf structure (ACT-bound: the exp stream at (N+352)/1.2 ns + 182
ns/READ_ACC per chunk is the hard wall, ~42us/core; PE warm ~38us):
  - NO xkvt input: the host pair-swaps adjacent 128-col tiles of x^T on
    odd cores so local j-tile r sits at xT[:, 256r:256r+128] on every
    core (SPMD-uniform); saves 512KB of the input-DMA wall.  The odd
    cores' att output comes back column-permuted; the host un-permutes.
  - causal mask applied ON THE PE via an identity-weights matmul that
    accumulates a -1e9 block into the scores PSUM (start=False) -- the
    vector engine is fully out of the PE->ACT critical path
  - the scalar queue carries only the exp stream + the two earliest xT
    DMA descriptors (issued before the stream starts)
  - row 0 is split into 5 chunks emitted in input-piece-landing order,
    so the exp stream starts as soon as the first 128KB piece lands
  - PE warmed with dummy matmuls during the input-DMA wall so the HAM
    clock gate (1.2 -> 2.4 GHz after ~3.4us of activity) flips before
    real work starts
  - PV matmuls run as a pending-thunk queue drained with a time budget
    after every activate, so they fill PE slack under the ACT stream;
    every output chunk opens as an early PSUM group (or SBUF partial
    for chunks 5-7) and closes 2 rows later; chunk 7's high half closes
    after row 15's single 256-col matmul + one merge
"""

import numpy as np

B, S, D = 4, 4096, 128
KD, VD = 64, 128
P = 128
NCORES = 8
JT = 16           # local j-tiles per core
CHUNK = 1536      # score chunk width (PSUM cols, 3 banks)

ROW_W = [S - 2 * i * P for i in range(JT)]          # E row widths
EOFF = [0] * JT
for _i in range(1, JT):
    EOFF[_i] = EOFF[_i - 1] + ROW_W[_i - 1]
ECOLS = EOFF[-1] + ROW_W[-1]                        # 34816

_CACHE = {}


def _build_program():
    from contextlib import ExitStack

    from concourse import bacc, mybir
    from concourse import tile as tile_mod

    dt = mybir.dt
    f32, bf16 = dt.float32, dt.bfloat16
    Alu = mybir.AluOpType
    ActF = mybir.ActivationFunctionType

    nc = bacc.Bacc(
        "TRN2", target_bir_lowering=False, debug=False, num_devices=NCORES
    )

    # xt: x^T with adjacent 128-col pairs SWAPPED on odd-parity cores, so
    # that core-local j-tile r always sits at columns [256r, 256r+128).
    # Scores/E/att then come out column-permuted; the host un-permutes.
    xt_d = nc.dram_tensor("xt", [P, S], bf16, kind="ExternalInput").ap()
    # s16: Mt[0:128] | WvT[128:256] | id[256:384] | mrow[384:640]
    s16_d = nc.dram_tensor("s16", [P, 640], bf16, kind="ExternalInput").ap()
    # s32: a[0] | bvb[1:129]
    s32_d = nc.dram_tensor("s32", [P, 129], f32, kind="ExternalInput").ap()
    att_d = nc.dram_tensor("att", [P, S], bf16, kind="ExternalOutput").ap()

    with tile_mod.TileContext(nc) as tc, ExitStack() as ctx:
        persist = ctx.enter_context(tc.tile_pool(name="persist", bufs=1))

        xT = persist.tile([P, S], bf16)            # [d, q'] (pair-swapped)
        GT = persist.tile([P, JT * P], bf16)       # [d, local j] = M xkv^T + a
        E_all = persist.tile([P, ECOLS], bf16)     # exp(scores^T) rows
        Vp = persist.tile([P, JT, VD], bf16)       # [j, v] scaled by 1/l
        l_all = persist.tile([P, JT], f32)
        linv = persist.tile([P, JT], f32)
        lp2 = persist.tile([P, JT], f32)           # chunk-1 l partials
        lp3 = persist.tile([P, JT], f32)           # chunk-2 l partials
        lp4 = persist.tile([P, JT], f32)           # chunk-3 l partials (row 0)
        lp5 = persist.tile([P, JT], f32)           # chunk-4 l partials (row 0)
        o7a = persist.tile([P, 512], f32)          # PV chunk-7 early partial
        V_sb = persist.tile([P, JT, VD], f32)      # V + bv, unscaled
        o5a = persist.tile([P, 512], f32)          # PV chunk-5 early partial
        o6a = persist.tile([P, 512], f32)          # PV chunk-6 early partial
        warm = persist.tile([P, 8], f32)           # exp-table warmup scratch
        scr = persist.tile([P, P], bf16)           # PE-warmup operand
        s16 = persist.tile([P, 640], bf16)
        s32 = persist.tile([P, 129], f32)
        a_sb = s32[:, 0:1]
        bvb = s32[:, 1:129]
        Mt = s16[:, 0:128]
        WvT = s16[:, 128:256]
        idm = s16[:, 256:384]
        mrow = s16[:, 384:640]

        # ---- PE/ACT warmup + input DMAs.  The scalar queue stays clean
        # (only the exp-table warmup); descriptors cost ~620ns each on the
        # issuing engine, so they're spread over sync/gpsimd/vector/tensor
        # in the order the pipeline first needs each piece.
        nc.gpsimd.memset(scr, 0.0)
        nc.gpsimd.memset(warm, 0.0)

        nc.sync.dma_start(out=s16, in_=s16_d)
        nc.gpsimd.dma_start(out=xT[:, 2048:3072], in_=xt_d[:, 2048:3072])
        nc.scalar.dma_start(out=xT[:, 0:512], in_=xt_d[:, 0:512])
        nc.scalar.dma_start(out=xT[:, 512:1024], in_=xt_d[:, 512:1024])
        nc.sync.dma_start(out=s32, in_=s32_d)
        nc.gpsimd.dma_start(out=xT[:, 3072:4096], in_=xt_d[:, 3072:4096])
        nc.sync.dma_start(out=xT[:, 1024:2048], in_=xt_d[:, 1024:2048])

        # exp-table load rides the scalar queue after its two descriptors
        nc.scalar.activation(out=warm, in_=warm, func=ActF.Exp)

        with ExitStack() as ph:
            ps = ph.enter_context(
                tc.tile_pool(name="ps", bufs=2, space="PSUM")
            )
            aux = ph.enter_context(
                tc.tile_pool(name="aux", bufs=2, space="PSUM")
            )
            osb = ph.enter_context(tc.tile_pool(name="osb", bufs=4))

            # dummy matmuls on memset scratch: ~2us of PE activity during
            # the DMA wall flips the HAM clock gate to 2.4 GHz before the
            # real stream begins
            wps = aux.tile([P, P], f32, tag="aux", name="warm_mm")
            for _ in range(20):
                nc.tensor.matmul(
                    wps, lhsT=scr, rhs=scr, start=True, stop=True
                )

            def emit_gt(r0, r1):
                # G tiles for local j-tiles [r0, r1): tile r lives at
                # xT[:, 256r : 256r+128] under the pair-swapped layout
                pgt = aux.tile(
                    [P, (r1 - r0) * P], f32, tag="aux", name=f"gt_{r0}"
                )
                for r in range(r0, r1):
                    nc.tensor.matmul(
                        pgt[:, (r - r0) * P : (r - r0 + 1) * P],
                        lhsT=Mt,
                        rhs=xT[:, 256 * r : 256 * r + P],
                        start=True,
                        stop=True,
                    )
                nc.vector.tensor_scalar(
                    out=GT[:, r0 * P : r1 * P],
                    in0=pgt,
                    scalar1=a_sb,
                    scalar2=None,
                    op0=Alu.add,
                )

            # ---- pending PV work: thunks drained between score chunks so
            # the PV matmuls fill the PE slack while ACT owns the pace
            pending = []

            def drain(budget):
                while pending and budget > 0:
                    est, fn = pending.pop(0)
                    fn()
                    budget -= est

            def chunk_bounds(i):
                # row 0's chunks align with the input DMA pieces so each
                # activate is gated by exactly one landing transfer
                if i == 0:
                    return [0, 512, 1024, 2048, 3072, 4096]
                w = ROW_W[i]
                return list(range(0, w, CHUNK)) + [w]

            def emit_chunk(i, ci):
                bounds = chunk_bounds(i)
                q0 = 256 * i
                c0, cw = bounds[ci], bounds[ci + 1] - bounds[ci]
                sc = ps.tile([P, CHUNK], f32, tag="ps", name=f"sc_{i}_{ci}")
                for s0 in range(0, cw, 512):
                    sw = min(512, cw - s0)
                    off = q0 + c0 + s0
                    nc.tensor.matmul(
                        sc[:, s0 : s0 + sw],
                        lhsT=GT[:, i * P : (i + 1) * P],
                        rhs=xT[:, off : off + sw],
                        start=True,
                        stop=True,
                    )
                    if ci == 0 and s0 == 0:
                        # accumulate the -1e9 diagonal mask block on
                        # the PE itself (identity weights, mask rhs)
                        nc.tensor.matmul(
                            sc[:, : 2 * P],
                            lhsT=idm,
                            rhs=mrow,
                            start=False,
                            stop=True,
                            skip_group_check=True,
                        )
                ecol = EOFF[i] + c0
                nc.scalar.activation(
                    out=E_all[:, ecol : ecol + cw],
                    in_=sc[:, :cw],
                    func=ActF.Exp,
                    accum_out=[l_all, lp2, lp3, lp4, lp5][ci][:, i : i + 1],
                )
                b = int(0.55 * cw) + 280
                if i in (3, 5, 7) and ci == 0:
                    b -= 700
                drain(b)

            def finish_row(i):
                nch = len(chunk_bounds(i)) - 1
                for pp in ([lp2, lp3, lp4, lp5][: nch - 1]):
                    nc.vector.tensor_tensor(
                        out=l_all[:, i : i + 1],
                        in0=l_all[:, i : i + 1],
                        in1=pp[:, i : i + 1],
                        op=Alu.add,
                    )
                nc.vector.reciprocal(linv[:, i : i + 1], l_all[:, i : i + 1])
                nc.vector.tensor_scalar(
                    out=Vp[:, i, :],
                    in0=V_sb[:, i, :],
                    scalar1=linv[:, i : i + 1],
                    scalar2=None,
                    op0=Alu.mult,
                )

            def emit_qk_row(i):
                for ci in range(len(chunk_bounds(i)) - 1):
                    emit_chunk(i, ci)
                finish_row(i)

            def emit_v(i):
                # V projection for tile i (runs at startup; needs no l)
                pv = aux.tile([P, VD], f32, tag="aux", name=f"v_{i}")
                nc.tensor.matmul(
                    pv,
                    lhsT=xT[:, 256 * i : 256 * i + P],
                    rhs=WvT,
                    start=True,
                    stop=True,
                )
                nc.vector.tensor_tensor(
                    out=V_sb[:, i, :], in0=pv, in1=bvb, op=Alu.add
                )

            OUTQ = [nc.sync, nc.gpsimd]

            def emit_out(c, ap, merge=None):
                ob = osb.tile([P, 512], bf16, tag="osb", name=f"osb_{c}")
                if merge is None:
                    nc.vector.tensor_copy(ob, ap)
                else:
                    nc.vector.tensor_tensor(
                        out=ob, in0=ap, in1=merge, op=Alu.add
                    )
                OUTQ[c % 2].dma_start(
                    out=att_d[:, c * 512 : (c + 1) * 512], in_=ob
                )

            class PvGroup:
                """PSUM accumulation over rows for output cols
                [g0, g0+gw); MMs are enqueued as thunks and drained
                between score chunks."""

                def __init__(self, name, g0, gw, shared=None, half=None):
                    self.name, self.g0, self.gw = name, g0, gw
                    self.shared, self.half = shared, half
                    self.ap = None
                    self.started = False

                def _ap(self):
                    if self.shared is not None:
                        if self.shared.get("ap") is None:
                            self.shared["ap"] = aux.tile(
                                [P, 512], f32, tag="aux", name=self.name
                            )
                        full = self.shared["ap"]
                        h = self.half
                        return full[:, h * 256 : h * 256 + 256]
                    if self.ap is None:
                        self.ap = aux.tile(
                            [P, self.gw], f32, tag="aux", name=self.name
                        )
                    return self.ap

                def enq(self, rows, last=False):
                    rows = list(rows)
                    for k, ii in enumerate(rows):
                        pending.append(
                            self._mk(ii, last and k == len(rows) - 1)
                        )

                def _mk(self, ii, is_last):
                    g0, gw = self.g0, self.gw
                    lo_q = max(g0, 256 * ii)
                    n = g0 + gw - lo_q
                    ecol = EOFF[ii] + lo_q - 256 * ii
                    p0 = lo_q - g0

                    def fn():
                        ap = self._ap()
                        st = not self.started
                        self.started = True
                        nc.tensor.matmul(
                            ap[:, p0 : p0 + n],
                            lhsT=Vp[:, ii, :],
                            rhs=E_all[:, ecol : ecol + n],
                            start=st,
                            stop=is_last,
                            skip_group_check=True,
                        )

                    return (int(n * 0.42) + 25, fn)

                def fin(self, mode, other=None, lo=0, hi=None):
                    # mode: "out" -> copy+DMA, "save" -> copy to SBUF
                    # partial, "merge" -> add SBUF partial, then DMA.
                    # lo/hi select a column sub-range of the group.
                    def fn():
                        h = self.gw if hi is None else hi
                        ap = self._ap()[:, lo:h]
                        g0, gw = self.g0 + lo, h - lo
                        if mode == "save":
                            nc.vector.tensor_copy(other, ap)
                            return
                        ob = osb.tile(
                            [P, gw], bf16, tag="osb", name=f"ob_{self.name}_{lo}"
                        )
                        if mode == "merge":
                            nc.vector.tensor_tensor(
                                out=ob, in0=ap, in1=other, op=Alu.add
                            )
                        else:
                            nc.vector.tensor_copy(ob, ap)
                        OUTQ[(g0 // 512 + (g0 % 512) // 256) % 2].dma_start(
                            out=att_d[:, g0 : g0 + gw], in_=ob
                        )

                    pending.append((80, fn))

            g7t = PvGroup("pv7t", 3584, 512)
            g5p = PvGroup("pv5p", 2560, 512)
            g6p = PvGroup("pv6p", 3072, 512)
            g7p = PvGroup("pv7p", 3584, 512)
            gful = {}

            # ---- early phase: row 0's chunks emitted in the order their
            # xT pieces land (piece 3 arrives on the lightly-loaded gpsimd
            # ring before piece 2), so the exp stream never starves while
            # the 1MB of x^T is still in flight
            emit_gt(0, 1)
            emit_v(0)
            emit_chunk(0, 0)
            emit_gt(1, 3)
            emit_v(1)
            emit_v(2)
            for ci in (1, 3, 2, 4):
                emit_chunk(0, ci)
            finish_row(0)

            for i in range(1, JT):
                if i == 1:
                    emit_gt(3, 7)
                    emit_v(3)
                if i == 3:
                    emit_gt(7, 11)
                    for t in range(4, 8):
                        emit_v(t)
                if i == 5:
                    emit_gt(11, 14)
                    for t in range(8, 12):
                        emit_v(t)
                if i == 7:
                    emit_gt(14, 16)
                    for t in range(12, 16):
                        emit_v(t)
                emit_qk_row(i)
                # PV schedule: every chunk opens as soon as most of its
                # rows exist and closes two rows later (open PSUM groups,
                # FIFO drain order keeps at most 2 groups live); SBUF
                # partials front-load chunks 5-7; chunk 7's last rows
                # accumulate in an open group so only row 15's MM + one
                # merge trail the exp stream
                if i == 1:
                    gful[0] = PvGroup("pv0", 0, 512)
                    gful[0].enq(range(0, 2), last=True)
                    gful[0].fin("out")
                    gful[1] = PvGroup("pv1", 512, 512)
                    gful[1].enq(range(0, 2))
                if i == 3:
                    gful[1].enq(range(2, 4), last=True)
                    gful[1].fin("out")
                    gful[2] = PvGroup("pv2", 1024, 512)
                    gful[2].enq(range(0, 4))
                if i == 5:
                    gful[2].enq(range(4, 6), last=True)
                    gful[2].fin("out")
                    gful[3] = PvGroup("pv3", 1536, 512)
                    gful[3].enq(range(0, 6))
                    g5p.enq(range(0, 6))
                if i == 7:
                    gful[3].enq(range(6, 8), last=True)
                    gful[3].fin("out")
                    g5p.enq(range(6, 8), last=True)
                    g5p.fin("save", o5a)
                    gful[4] = PvGroup("pv4", 2048, 512)
                    gful[4].enq(range(0, 8))
                    g6p.enq(range(0, 8))
                if i == 9:
                    gful[4].enq(range(8, 10), last=True)
                    gful[4].fin("out")
                    g6p.enq(range(8, 10), last=True)
                    g6p.fin("save", o6a)
                    g7p.enq(range(0, 10))
                if i == 11:
                    g7p.enq(range(10, 12), last=True)
                    g7p.fin("save", o7a)
                    g5t = PvGroup("pv5t", 2560, 512)
                    g5t.enq(range(8, 12), last=True)
                    g5t.fin("merge", o5a)
                    g6t = PvGroup("pv6t", 3072, 512)
                    g6t.enq(range(10, 12))
                if i == 13:
                    g6t.enq(range(12, 14), last=True)
                    g6t.fin("merge", o6a)
                    g7t.enq(range(12, 14))
                if i == 14:
                    # after row 14, output cols [3584:3840] are final
                    g7t.enq([14])
                    g7t.fin("merge", o7a[:, 0:256], lo=0, hi=256)
                if i == 15:
                    g7t.enq([15], last=True)
                    g7t.fin("merge", o7a[:, 256:512], lo=256, hi=512)
            drain(10**9)

    nc.compile()
    return nc


def _host_inputs(x, Wq, bq, Wk, bk, Wv, bv):
    """Per-core input maps (host does layout prep + tiny precomputes)."""
    import ml_dtypes

    hf = ml_dtypes.bfloat16
    x_full = np.ascontiguousarray(x, dtype=np.float32)
    Wq = np.asarray(Wq, np.float32)
    Wk = np.asarray(Wk, np.float32)
    bk = np.asarray(bk, np.float32)
    Wv = np.asarray(Wv, np.float32)
    bv = np.asarray(bv, np.float32)

    M = (Wq.T @ Wk) / 8.0                      # [D, D]
    Mt = np.ascontiguousarray(M.T).astype(hf)
    a = ((Wq.T @ bk) / 8.0).reshape(D, 1)      # [D, 1]
    WvT = np.ascontiguousarray(Wv.T).astype(hf)
    bvb = np.tile(bv.reshape(1, VD), (P, 1))   # [P, V]
    idm = np.eye(P, dtype=np.float32)

    # mask row: diagonal tile is ALWAYS the first 128 cols of a row under
    # the pair-swapped layout; for p=1 the second 128 cols are the
    # lower-numbered global tile -> fully masked
    tri = np.where(
        np.arange(P)[None, :] >= np.arange(P)[:, None], 0.0, -1e9
    ).astype(np.float32)
    mrows = []
    for p in (0, 1):
        m = np.zeros((P, 2 * P), np.float32)
        m[:, :P] = tri
        if p == 1:
            m[:, P:] = -1e9
        mrows.append(m)

    s16s = [
        np.ascontiguousarray(
            np.concatenate([Mt, WvT, idm.astype(hf), mrows[p].astype(hf)],
                           axis=1)
        )
        for p in (0, 1)
    ]
    s32 = np.ascontiguousarray(
        np.concatenate([a, bvb], axis=1).astype(np.float32)
    )
    # per-parity xT: odd cores get adjacent 128-col pairs swapped so local
    # j-tile r sits at columns [256r, 256r+128) on every core
    sw = np.arange(S // P).reshape(-1, 2)[:, ::-1].reshape(-1)
    xts = []
    for b in range(B):
        xt = np.ascontiguousarray(x_full[b].T.astype(hf))
        xts.append(
            (
                xt,
                np.ascontiguousarray(
                    xt.reshape(P, S // P, P)[:, sw, :].reshape(P, S)
                ),
            )
        )
    in_maps = []
    for c in range(NCORES):
        b, p = c // 2, c % 2
        in_maps.append(
            {
                "xt": xts[b][p],
                "s16": s16s[p],
                "s32": s32,
            }
        )
    return in_maps


def _get_program():
    if "nc" not in _CACHE:
        _CACHE["nc"] = _build_program()
    return _CACHE["nc"]


def run_on_device(in_maps, trace=False, trace_kwargs=None):
    from concourse import bass_utils

    nc = _get_program()
    return bass_utils.run_bass_kernel_spmd(
        nc,
        in_maps,
        core_ids=list(range(NCORES)),
        trace=trace,
        trace_kwargs=trace_kwargs or {},
    )


def kernel(x, Wq, bq, Wk, bk, Wv, bv):
    x = np.asarray(x, np.float32)
    in_maps = _host_inputs(x, Wq, bq, Wk, bk, Wv, bv)
    res = run_on_device(in_maps)
    sw = np.arange(S // P).reshape(-1, 2)[:, ::-1].reshape(-1)
    out = np.empty((B, S, D + VD), np.float32)
    for b in range(B):
        a1 = res.results[2 * b + 1]["att"].astype(np.float32)
        # odd core's att columns are pair-swapped; undo before the add
        a1 = a1.reshape(P, S // P, P)[:, sw, :].reshape(P, S)
        attT = res.results[2 * b]["att"].astype(np.float32) + a1
        out[b, :, :D] = x[b]
        out[b, :, D:] = attT.T
    return out
